# revision 8
# baseline (speedup 1.0000x reference)
"""Trainium2 Bass kernel for nn_AggregateConcatenate.

out[b] = concat([masked {mean,max,min,std} of tanh-MLP_agg(x_b) over the valid
prefix, tanh-MLP_adj(x_b)], axis=1)

Sharding: data-parallel over batch B across 8 NeuronCores (4 bags/core).

Per-core dataflow (matmuls in float32r: full PE rate at N>=256, ~1.5e-4 rel
error — 16x tighter than bf16):
  - mm1 feature-major: h^T[h, tok] = W1T.T @ x^T   (x pre-transposed on host)
  - ELU(pre) = relu(pre) + min(exp(pre) - 1, 0): exp+relu on ScalarE (bias b1
    fused per h-tile), combine on VectorE.
  - mm2 token-major: q[tok, a] = hT.T @ W2T  -> adjacent output rows DMA out
    contiguously; the ragged reductions see tokens on partitions.
  - masked sum / sum-of-squares via PE matmuls with per-(bag, tile) mask
    columns as the stationary operand, accumulated in PSUM across the whole
    kernel; max/min via VectorE candidates (mask * q + (mask-1)*BIG) into
    per-bag accumulators, reduced over partitions at the end with PE
    transposes.
"""

import numpy as np

import concourse.bass as bass
import concourse.tile as tile
from concourse import mybir
from concourse.bass_utils import run_bass_kernel_spmd
from concourse.masks import make_identity
from concourse.tile import ScopedClock

B, T, E, H, A = 32, 2048, 512, 1024, 512
NCORES = 8
BPC = B // NCORES          # bags per core
NCH = T // 512             # 512-token chunks per bag
NTT = 4                    # 128-token tiles per chunk
KE = E // 128              # k-tiles for mm1
KH = H // 128              # k-tiles for mm2
NHT = H // 128             # h-tiles (partition tiles of H)
BIG = 30.0                 # ragged padding offset; |tanh| < 1 << BIG

F32 = mybir.dt.float32
F32R = mybir.dt.float32r
AF = mybir.ActivationFunctionType
OP = mybir.AluOpType


class _SplitDrainTileContext(tile.TileContext):
    """TileContext whose exit drain splits sem waits across sync NOPs."""

    WAIT_LIMIT = 1

    def _drain_and_barrier(self, tick_clock, wait_clock):
        drain_inst = self.nc.sync.drain()
        wait_clock.add_sem_waits(
            drain_inst.ins, ScopedClock({None: tick_clock.global_clock})
        )
        si = drain_inst.ins.sync_info
        if si is not None and len(si.on_wait) > self.WAIT_LIMIT:
            waits = list(si.on_wait)
            drain_inst.ins.sync_info = mybir.SyncInfo(
                on_wait=waits[: self.WAIT_LIMIT], on_update=list(si.on_update)
            )
            for i in range(self.WAIT_LIMIT, len(waits), self.WAIT_LIMIT):
                nop = self.nc.sync.nop()
                nop.ins.sync_info = mybir.SyncInfo(
                    on_wait=waits[i : i + self.WAIT_LIMIT], on_update=[]
                )
        self.nc.all_engine_barrier()
        assert self.sems is not None
        popped = self.nc._tile_sem_poison_stack.pop()
        assert popped is self._sem_poison
        self.nc.clear_and_free_semaphores(list(self.sems.allocated().values()))
        self.nc.all_engine_barrier()


def _split_waits(nc, limit: int = 1):
    """Walrus codegen accepts at most one sync wait per TPB instruction.

    Hoist excess waits from any instruction onto injected same-engine NOPs
    placed immediately before it (same-engine program order is preserved, so
    waiting earlier is equivalent).
    """
    uid = [0]
    for f in nc.m.functions:
        for bb in f.blocks:
            new_insts = []
            for ins in bb.instructions:
                si = ins.sync_info
                if si is not None and len(si.on_wait) > limit:
                    waits = list(si.on_wait)
                    keep = waits[:limit]
                    rest = waits[limit:]
                    for j in range(0, len(rest), limit):
                        uid[0] += 1
                        nop = mybir.InstNoOp(
                            name=f"waitnop-{uid[0]}",
                            engine=ins.engine,
                            ins=[],
                            outs=[],
                        )
                        nop.sync_info = mybir.SyncInfo(
                            on_wait=rest[j : j + limit], on_update=[]
                        )
                        new_insts.append(nop)
                    ins.sync_info = mybir.SyncInfo(
                        on_wait=keep, on_update=list(si.on_update)
                    )
                new_insts.append(ins)
            if len(new_insts) != len(bb.instructions):
                bb.instructions = new_insts
    return nc


def _build_program(with_b2: bool, b1_zero: bool):
    nc = bass.Bass()

    xt = nc.declare_dram_parameter("xt", [BPC, E, T], F32R, isOutput=False)
    w1t = {}
    w2t = {}
    b1 = {}
    b2 = {}
    for m in ("agg", "adj"):
        w1t[m] = nc.declare_dram_parameter(f"w1t_{m}", [E, H], F32R, isOutput=False)
        w2t[m] = nc.declare_dram_parameter(f"w2t_{m}", [H, A], F32R, isOutput=False)
        b1[m] = nc.declare_dram_parameter(f"b1_{m}", [H], F32, isOutput=False)
        if with_b2:
            b2[m] = nc.declare_dram_parameter(f"b2_{m}", [1, A], F32R, isOutput=False)
    maskd = nc.declare_dram_parameter("mask", [BPC, T], F32, isOutput=False)
    negmaskd = nc.declare_dram_parameter("negmask", [BPC, T], F32, isOutput=False)
    negpadd = nc.declare_dram_parameter("negpad", [BPC, T], F32, isOutput=False)
    slhsd = nc.declare_dram_parameter("slhs", [BPC, 16, 128, BPC], F32R, isOutput=False)
    invnd = nc.declare_dram_parameter("inv_n", [BPC, 1], F32, isOutput=False)
    invnm1d = nc.declare_dram_parameter("inv_nm1", [BPC, 1], F32, isOutput=False)
    novernm1d = nc.declare_dram_parameter("n_over_nm1", [BPC, 1], F32, isOutput=False)
    out = nc.declare_dram_parameter("out", [BPC, 4 + T, A], F32, isOutput=True)

    with _SplitDrainTileContext(nc) as tc:
        with (
            tc.tile_pool(name="consts", bufs=1) as consts,
            tc.tile_pool(name="accs", bufs=1) as accs,
            tc.tile_pool(name="xin", bufs=2) as xin,
            tc.tile_pool(name="elu", bufs=2) as elu,
            tc.tile_pool(name="ht", bufs=1) as htp,
            tc.tile_pool(name="qp", bufs=1) as qp,
            tc.tile_pool(name="adjo", bufs=3) as adjo,
            tc.tile_pool(name="fin", bufs=1) as fin,
            tc.tile_pool(name="pb1", bufs=(2 if b1_zero else 4), space="PSUM") as pb1p,
            tc.tile_pool(name="pb2", bufs=2, space="PSUM") as pb2p,
            tc.tile_pool(name="pstat", bufs=1, space="PSUM") as pstat,
        ):
            # ---- constants -------------------------------------------------
            w1sb = {}
            w2sb = {}
            b1sb = {}
            b2sb = {}
            for m in ("agg", "adj"):
                w1sb[m] = consts.tile([128, KE, H], F32R, tag=f"w1_{m}", name=f"w1_{m}")
                nc.sync.dma_start(
                    out=w1sb[m], in_=w1t[m][:, :].rearrange("(kt p) h -> p kt h", p=128)
                )
                w2sb[m] = consts.tile([128, KH, A], F32R, tag=f"w2_{m}", name=f"w2_{m}")
                nc.sync.dma_start(
                    out=w2sb[m], in_=w2t[m][:, :].rearrange("(kt p) a -> p kt a", p=128)
                )
                b1sb[m] = consts.tile([128, NHT], F32, tag=f"b1_{m}", name=f"b1_{m}")
                nc.sync.dma_start(
                    out=b1sb[m], in_=b1[m][:].rearrange("(j p) -> p j", p=128)
                )
                if with_b2:
                    b2sb[m] = consts.tile([1, A], F32R, tag=f"b2_{m}", name=f"b2_{m}")
                    nc.sync.dma_start(out=b2sb[m], in_=b2[m][:, :])
            if with_b2:
                ones_col = consts.tile([1, 128], F32R, tag="ones", name="ones")
                nc.gpsimd.memset(ones_col, 1.0)

            mask_sb = consts.tile([128, BPC, 16], F32, tag="mask", name="mask")
            nc.sync.dma_start(
                out=mask_sb, in_=maskd[:, :].rearrange("b (tt p) -> p b tt", p=128)
            )
            negmask_sb = consts.tile([128, BPC, 16], F32, tag="negmask", name="negmask")
            nc.sync.dma_start(
                out=negmask_sb, in_=negmaskd[:, :].rearrange("b (tt p) -> p b tt", p=128)
            )
            negpad_sb = consts.tile([128, BPC, 16], F32, tag="negpad", name="negpad")
            nc.sync.dma_start(
                out=negpad_sb, in_=negpadd[:, :].rearrange("b (tt p) -> p b tt", p=128)
            )
            slhs_sb = consts.tile([128, BPC, 16, BPC], F32R, tag="slhs", name="slhs")
            nc.sync.dma_start(
                out=slhs_sb, in_=slhsd[:, :, :, :].rearrange("b tt p c -> p b tt c")
            )
            invn_sb = consts.tile([BPC, 1], F32, tag="invn", name="invn")
            nc.sync.dma_start(out=invn_sb, in_=invnd[:, :])
            invnm1_sb = consts.tile([BPC, 1], F32, tag="invnm1", name="invnm1")
            nc.sync.dma_start(out=invnm1_sb, in_=invnm1d[:, :])
            novernm1_sb = consts.tile([BPC, 1], F32, tag="novernm1", name="novernm1")
            nc.sync.dma_start(out=novernm1_sb, in_=novernm1d[:, :])

            ident_f = consts.tile([128, 128], F32, tag="ident_f", name="ident_f")
            make_identity(nc, ident_f)

            # per-bag running accumulators for max(q) and max(-q)
            acc_max = accs.tile([128, BPC, A], F32, tag="acc_max", name="acc_max")
            nc.gpsimd.memset(acc_max, -1e4)
            acc_nmax = accs.tile([128, BPC, A], F32, tag="acc_nmax", name="acc_nmax")
            nc.gpsimd.memset(acc_nmax, -1e4)

            # stats accumulators in PSUM, one matmul accumulation group each
            psum_s = pstat.tile([BPC, A], F32, tag="psum_s", name="psum_s")
            psum_q = pstat.tile([BPC, A], F32, tag="psum_q", name="psum_q")
            n_stat_mm = BPC * NCH * NTT  # matmuls per accumulation group

            # ---- main loops ------------------------------------------------
            stat_i = 0
            for b in range(BPC):
                for c in range(NCH):
                    ts = slice(c * 512, (c + 1) * 512)
                    xb = xin.tile([128, KE, 512], F32R, tag="xb", name="xb")
                    nc.sync.dma_start(
                        out=xb,
                        in_=xt[b, :, ts].rearrange("(kt p) t -> p kt t", p=128),
                    )
                    hts = {}
                    # ---- layer 1 (feature-major) + ELU, both MLPs ----
                    # elu(z) = max(z, min(exp(z) - 1, 0)); the max runs as one
                    # fused scalar_tensor_tensor on VectorE reading PSUM.
                    for m in ("agg", "adj"):
                        htsb = htp.tile(
                            [128, KH, 512], F32R, tag=f"ht_{m}", name=f"ht_{m}"
                        )
                        hts[m] = htsb
                        if b1_zero:
                            for hg in range(NHT // 2):
                                pb1 = pb1p.tile(
                                    [128, 2, 512], F32, tag="pb1", name="pb1"
                                )
                                for j in range(2):
                                    ht = hg * 2 + j
                                    for kt in range(KE):
                                        nc.tensor.matmul(
                                            pb1[:, j, :],
                                            lhsT=w1sb[m][:, kt, ht * 128 : (ht + 1) * 128],
                                            rhs=xb[:, kt, :],
                                            start=(kt == 0),
                                            stop=(kt == KE - 1),
                                        )
                                e_sb = elu.tile([128, 2, 512], F32, tag="e", name="e")
                                nc.scalar.activation(e_sb, pb1, AF.Exp)
                                m_sb = elu.tile([128, 2, 512], F32, tag="m", name="m")
                                nc.gpsimd.tensor_scalar(
                                    m_sb, e_sb, -1.0, 0.0, OP.add, OP.min
                                )
                                nc.vector.scalar_tensor_tensor(
                                    out=htsb[:, hg * 2 : hg * 2 + 2, :],
                                    in0=pb1, scalar=0.0, in1=m_sb,
                                    op0=OP.add, op1=OP.max,
                                )
                        else:
                            for ht in range(NHT):
                                pb1 = pb1p.tile([128, 512], F32, tag="pb1", name="pb1")
                                for kt in range(KE):
                                    nc.tensor.matmul(
                                        pb1,
                                        lhsT=w1sb[m][:, kt, ht * 128 : (ht + 1) * 128],
                                        rhs=xb[:, kt, :],
                                        start=(kt == 0),
                                        stop=(kt == KE - 1),
                                    )
                                b1col = b1sb[m][:, ht : ht + 1]
                                e_sb = elu.tile([128, 512], F32, tag="e", name="e")
                                nc.scalar.activation(e_sb, pb1, AF.Exp, bias=b1col)
                                m_sb = elu.tile([128, 512], F32, tag="m", name="m")
                                nc.gpsimd.tensor_scalar(
                                    m_sb, e_sb, -1.0, 0.0, OP.add, OP.min
                                )
                                nc.vector.scalar_tensor_tensor(
                                    out=htsb[:, ht, :],
                                    in0=pb1, scalar=b1col, in1=m_sb,
                                    op0=OP.add, op1=OP.max,
                                )
                    # ---- layer 2 (token-major) + tanh ----
                    q_sb = qp.tile([128, NTT, A], F32R, tag="q", name="q")
                    q2_sb = qp.tile([128, NTT, A], F32R, tag="q2", name="q2")
                    for m in ("agg", "adj"):
                        for tt in range(NTT):
                            pb2 = pb2p.tile([128, A], F32, tag="pb2", name="pb2")
                            for kt in range(KH):
                                nc.tensor.matmul(
                                    pb2,
                                    lhsT=hts[m][:, kt, tt * 128 : (tt + 1) * 128],
                                    rhs=w2sb[m][:, kt, :],
                                    start=(kt == 0),
                                    stop=(kt == KH - 1) and not with_b2,
                                )
                            if with_b2:
                                nc.tensor.matmul(
                                    pb2, lhsT=ones_col, rhs=b2sb[m],
                                    start=False, stop=True,
                                )
                            if m == "adj":
                                adj_sb = adjo.tile([128, A], F32, tag="adj", name="adj")
                                nc.scalar.activation(adj_sb, pb2, AF.Tanh)
                                nc.sync.dma_start(
                                    out=out[
                                        b,
                                        4 + c * 512 + tt * 128 : 4 + c * 512 + (tt + 1) * 128,
                                        :,
                                    ],
                                    in_=adj_sb,
                                )
                            else:
                                nc.scalar.activation(q_sb[:, tt, :], pb2, AF.Tanh)
                                tg = c * NTT + tt
                                nc.gpsimd.tensor_tensor(
                                    q2_sb[:, tt, :], q_sb[:, tt, :],
                                    q_sb[:, tt, :], OP.mult,
                                )
                                cand = elu.tile([128, A], F32, tag="cand", name="cand")
                                nc.vector.tensor_scalar(
                                    cand, q_sb[:, tt, :],
                                    mask_sb[:, b, tg : tg + 1],
                                    negpad_sb[:, b, tg : tg + 1],
                                    OP.mult, OP.add,
                                )
                                nc.vector.tensor_tensor(
                                    acc_max[:, b, :], acc_max[:, b, :], cand, OP.max
                                )
                                cand2 = elu.tile([128, A], F32, tag="cand2", name="cand2")
                                nc.vector.tensor_scalar(
                                    cand2, q_sb[:, tt, :],
                                    negmask_sb[:, b, tg : tg + 1],
                                    negpad_sb[:, b, tg : tg + 1],
                                    OP.mult, OP.add,
                                )
                                nc.vector.tensor_tensor(
                                    acc_nmax[:, b, :], acc_nmax[:, b, :], cand2,
                                    OP.max,
                                )
                    # masked sum / sumsq, deferred so the PE never waits on q
                    for tt in range(NTT):
                        tg = c * NTT + tt
                        lhs = slhs_sb[:, b, tg, :]
                        nc.tensor.matmul(
                            psum_s, lhsT=lhs, rhs=q_sb[:, tt, :],
                            start=(stat_i == 0), stop=(stat_i == n_stat_mm - 1),
                            skip_group_check=True,
                        )
                        nc.tensor.matmul(
                            psum_q, lhsT=lhs, rhs=q2_sb[:, tt, :],
                            start=(stat_i == 0), stop=(stat_i == n_stat_mm - 1),
                            skip_group_check=True,
                        )
                        stat_i += 1

            # ---- finalize --------------------------------------------------
            # mean / std rows (partition = bag)
            mean_sb = fin.tile([BPC, A], F32, tag="mean", name="mean")
            nc.vector.tensor_scalar(mean_sb, psum_s, invn_sb[:, 0:1], None, OP.mult)
            m2_sb = fin.tile([BPC, A], F32, tag="m2", name="m2")
            nc.vector.tensor_tensor(m2_sb, mean_sb, mean_sb, OP.mult)
            s1_sb = fin.tile([BPC, A], F32, tag="s1", name="s1")
            nc.vector.tensor_scalar(s1_sb, psum_q, invnm1_sb[:, 0:1], None, OP.mult)
            s2_sb = fin.tile([BPC, A], F32, tag="s2", name="s2")
            nc.vector.tensor_scalar(s2_sb, m2_sb, novernm1_sb[:, 0:1], None, OP.mult)
            var_sb = fin.tile([BPC, A], F32, tag="var", name="var")
            nc.vector.tensor_tensor(var_sb, s1_sb, s2_sb, OP.subtract)
            std_sb = fin.tile([BPC, A], F32, tag="std", name="std")
            nc.scalar.activation(std_sb, var_sb, AF.Sqrt)
            for b in range(BPC):
                nc.sync.dma_start(out=out[b, 0:1, :], in_=mean_sb[b : b + 1, :])
                nc.sync.dma_start(out=out[b, 3:4, :], in_=std_sb[b : b + 1, :])

            # max / min rows: transpose accumulators and reduce over tokens
            for b in range(BPC):
                for acc, row, neg in ((acc_max, 1, False), (acc_nmax, 2, True)):
                    pt = pb1p.tile([128, NTT, 128], F32, tag="pb1", name="pt_fin")
                    redt = fin.tile(
                        [128, NTT], F32, tag="redt", name=f"redt_{b}_{row}"
                    )
                    for ch in range(NTT):
                        nc.tensor.transpose(
                            pt[:, ch, :], acc[:, b, ch * 128 : (ch + 1) * 128],
                            ident_f,
                        )
                        nc.vector.tensor_reduce(
                            redt[:, ch : ch + 1], pt[:, ch, :],
                            axis=mybir.AxisListType.X, op=OP.max,
                        )
                    prow = pb2p.tile([NTT, 128], F32, tag="pb2", name="prow_fin")
                    nc.tensor.transpose(prow, redt, ident_f)
                    row_sb = fin.tile([NTT, 128], F32, tag="row", name=f"row_{b}_{row}")
                    nc.scalar.mul(row_sb, prow, -1.0 if neg else 1.0)
                    nc.sync.dma_start(
                        out=out[b, row : row + 1, :].rearrange(
                            "o (c f) -> (o c) f", c=NTT
                        ),
                        in_=row_sb,
                    )
    _split_waits(nc)
    return nc


_PROGRAM_CACHE: dict = {}


def kernel(**inputs) -> np.ndarray:
    x = np.asarray(inputs["x"], np.float32)
    lengths = np.asarray(inputs["padding_lengths"]).astype(np.int64)
    agg_W1 = np.asarray(inputs["agg_W1"], np.float32)
    agg_b1 = np.asarray(inputs["agg_b1"], np.float32)
    agg_W2 = np.asarray(inputs["agg_W2"], np.float32)
    agg_b2 = np.asarray(inputs["agg_b2"], np.float32)
    adj_W1 = np.asarray(inputs["adj_W1"], np.float32)
    adj_b1 = np.asarray(inputs["adj_b1"], np.float32)
    adj_W2 = np.asarray(inputs["adj_W2"], np.float32)
    adj_b2 = np.asarray(inputs["adj_b2"], np.float32)

    with_b2 = bool(np.any(agg_b2) or np.any(adj_b2))
    b1_zero = not (np.any(agg_b1) or np.any(adj_b1))
    key = (with_b2, b1_zero)
    if key not in _PROGRAM_CACHE:
        _PROGRAM_CACHE[key] = _build_program(with_b2, b1_zero)
    nc = _PROGRAM_CACHE[key]

    # ---- host-side input prep ---------------------------------------------
    xt = np.ascontiguousarray(x.transpose(0, 2, 1))  # [B, E, T]
    w1t = {"agg": np.ascontiguousarray(agg_W1.T), "adj": np.ascontiguousarray(adj_W1.T)}
    w2t = {"agg": np.ascontiguousarray(agg_W2.T), "adj": np.ascontiguousarray(adj_W2.T)}
    b1 = {"agg": agg_b1, "adj": adj_b1}
    b2 = {"agg": agg_b2.reshape(1, A), "adj": adj_b2.reshape(1, A)}

    mask = (np.arange(T)[None, :] < lengths[:, None]).astype(np.float32)  # [B, T]
    negmask = -mask
    negpad = (mask - 1.0) * BIG
    # stationary mask columns for the stats matmuls: [B, 16 tok-tiles, 128, BPC]
    slhs = np.zeros((B, 16, 128, BPC), np.float32)
    mask_t = mask.reshape(B, 16, 128)
    for b in range(B):
        slhs[b, :, :, b % BPC] = mask_t[b]
    n = lengths.astype(np.float64)
    inv_n = (1.0 / n).astype(np.float32).reshape(B, 1)
    inv_nm1 = (1.0 / (n - 1.0)).astype(np.float32).reshape(B, 1)
    n_over_nm1 = (n / (n - 1.0)).astype(np.float32).reshape(B, 1)

    in_maps = []
    for c in range(NCORES):
        sl = slice(c * BPC, (c + 1) * BPC)
        im = {
            "xt": xt[sl],
            "w1t_agg": w1t["agg"], "w2t_agg": w2t["agg"], "b1_agg": b1["agg"],
            "w1t_adj": w1t["adj"], "w2t_adj": w2t["adj"], "b1_adj": b1["adj"],
            "mask": mask[sl],
            "negmask": negmask[sl],
            "negpad": negpad[sl],
            "slhs": slhs[sl],
            "inv_n": inv_n[sl], "inv_nm1": inv_nm1[sl],
            "n_over_nm1": n_over_nm1[sl],
        }
        if with_b2:
            im["b2_agg"] = b2["agg"]
            im["b2_adj"] = b2["adj"]
        in_maps.append(im)

    res = run_bass_kernel_spmd(nc, in_maps, core_ids=list(range(NCORES)))
    out = np.concatenate([res.results[c]["out"] for c in range(NCORES)], axis=0)
    return out.astype(np.float32)
